# revision 50
# baseline (speedup 1.0000x reference)
"""Pointer-generator attention kernel for 8 TRN2 NeuronCores.

Computation (per batch b):
    enc_feat = h[b] @ W_h.T                       # [T, N]
    att      = enc_feat + dec_fea[b] + cov[b,:,None] * W_c
    scores   = tanh(att) @ v                      # [T]
    attn     = exp(scores) * mask / sum(...)      # [T]
    c_t      = attn @ h[b]                        # [N]
    cov_new  = cov + attn

Sharding: data-parallel over batch, 8 batches per core, no collectives.

Device-side layout (per core):
    Pass A runs in fp8-e4m3 with DoubleRow perf mode (2 K-planes per
    matmul): hT8 [8, N, T] e4m3 and W_hT e4m3 (W pre-scaled by 16 so its
    ~N(0, 1/1024) entries stay in e4m3's normal range; the tanh
    activation applies scale=1/16 to undo it).  PSUM accumulates fp32.
    The fp8 quantization error is compensated by a rank-1 score-domain
    correction r[b, t] ~= cbar * v^T (W h - W8 h8)[b, t] (cbar = E[tanh'])
    precomputed on the host and folded into the softmax for free via
    exp(s + rc) * mask == exp(s) * (mask * exp(rc)) -- this brings the
    output rel-err from ~2.3e-2 down to ~1.3e-2.
    The cov[t] * W_c[m] fold runs on the DVE fused with the PSUM->SBUF
    eviction; tanh on ScalarE (bf16 att), v-dot as M=1 bf16 matmuls on
    PE, softmax on single-partition rows (exp has no overflow risk:
    |score| <= ||v||_1 ~ 26).
    Pass B (c_t) runs on the PE for EVERY batch: the unnormalized
    exp*mask row is bounced through DRAM into a [128, 8] column tile
    (a DRAM AP can supply the partition dim directly), then M=1 bf16
    matmuls against h in natural [t, n] layout accumulate c_t, with
    1/sum folded into the PSUM eviction.  This keeps the DVE off the
    critical path (it was the bottleneck at ~23us/batch with a
    DVE-based pass B; the PE absorbs it for ~4us/batch).
"""

import os
import sys

import numpy as np

sys.path.insert(0, "/opt/trn_rl_repo")

import concourse.bass as bass  # noqa: E402
import concourse.tile as tile  # noqa: E402
from concourse import mybir  # noqa: E402
from concourse.bass_utils import run_bass_kernel_spmd  # noqa: E402

B, T, N = 64, 1024, 1024
NCORES = 8
BL = B // NCORES  # 8 local batches per core
P = 128
KC = N // P  # 8 contraction chunks
MT = N // P  # 8 output row tiles
F32 = mybir.dt.float32
BF16 = mybir.dt.bfloat16
FP8 = mybir.dt.float8e4
AF = mybir.ActivationFunctionType
ALU = mybir.AluOpType
DR = mybir.MatmulPerfMode.DoubleRow

WSCALE = 16.0  # W_h pre-scale before e4m3 quantization
CBAR = 0.5  # E[tanh'(att)] used by the rank-1 fp8 correction

LAST_EXEC_NS = None
_NC_CACHE = None


def build_bass():
    nc = bass.Bass()

    hT8_h = nc.declare_dram_parameter("hT8", [BL, N, T], FP8, isOutput=False)
    hnat_h = nc.declare_dram_parameter("hnat", [BL, T, N], BF16, isOutput=False)
    cov_h = nc.declare_dram_parameter("cov", [BL, T], F32, isOutput=False)
    covb_h = nc.declare_dram_parameter("covb", [BL, T], BF16, isOutput=False)
    # mask_h actually carries mask * exp(rc): the rank-1 fp8 score
    # correction rc enters the softmax for free via
    # exp(s + rc) * mask == exp(s) * (mask * exp(rc)).
    mask_h = nc.declare_dram_parameter("mask", [BL, T], F32, isOutput=False)
    sT_h = nc.declare_dram_parameter("sT", [N, BL], BF16, isOutput=False)
    whT_h = nc.declare_dram_parameter("WhT8", [N, N], FP8, isOutput=False)
    dwT_h = nc.declare_dram_parameter("decWT", [N, N], BF16, isOutput=False)
    decb_h = nc.declare_dram_parameter("decb", [1, N], BF16, isOutput=False)
    wcT_h = nc.declare_dram_parameter("WcT", [P, KC], F32, isOutput=False)
    vcol_h = nc.declare_dram_parameter("vcol", [P, KC], BF16, isOutput=False)

    atn_bounce = nc.dram_tensor("atn_bounce", [BL, T], BF16)
    ct_out = nc.declare_dram_parameter("out_ct", [BL, N], F32, isOutput=True)
    attn_out = nc.declare_dram_parameter("out_attn", [BL, T], F32, isOutput=True)
    cov_out = nc.declare_dram_parameter("out_cov", [BL, T], F32, isOutput=True)

    with tile.TileContext(nc) as tc:
        with (
            tc.tile_pool(name="const", bufs=1) as const,
            tc.tile_pool(name="ht8", bufs=3) as ht8p,
            tc.tile_pool(name="hnat", bufs=3) as hnatp,
            tc.tile_pool(name="att", bufs=3) as attp,
            tc.tile_pool(name="rows", bufs=2) as rowp,
            tc.tile_pool(name="rows1", bufs=2) as rowp1,
            tc.tile_pool(name="bc", bufs=BL) as bcp,
            tc.tile_pool(name="acol", bufs=2) as acolp,
            tc.tile_pool(name="psA", bufs=2, space="PSUM") as psA,
            tc.tile_pool(name="psS", bufs=1, space="PSUM") as psS,
            tc.tile_pool(name="psF", bufs=1, space="PSUM") as psF,
        ):
            # ---- PE warm-up: dummy matmuls while the first DMAs land, so
            # the HAM clock gate reaches 2.4 GHz before real work starts ----
            ones_col = const.tile([1, P], BF16)  # lhsT for broadcasts
            nc.any.memset(ones_col[:], 1.0)
            warm_row = const.tile([1, 512], BF16)
            nc.any.memset(warm_row[:], 0.0)
            ps_w = psA.tile([P, T], F32, tag="psA")
            for _ in range(16):
                nc.tensor.matmul(
                    ps_w[:, 0:512], ones_col[:], warm_row[:],
                    start=True, stop=True,
                )

            # ---- constants (issue order matters: prologue inputs first) ----
            wh = const.tile([P, KC, N], FP8)  # [n%128, n//128, m], W.T * 16
            vcol = const.tile([P, KC], BF16)
            wcT = const.tile([P, KC], F32)  # 16*W_c[mt*128+p] per-part scalars
            covb_sb = const.tile([1, BL, T], BF16)  # cov rows (bf16)
            nc.sync.dma_start(out=covb_sb[:], in_=covb_h[:].unsqueeze(0))
            dec_feaT = const.tile([P, MT, BL], F32)  # dec_fea[m, b] bias layout

            cov_bc_all = []

            # ---- prologue: dec_fea = s_t_hat @ dec_W.T + dec_b  -> [b, m] ----
            with tc.tile_pool(name="prol", bufs=1) as prol:
                st = prol.tile([P, KC, BL], BF16, tag="st")
                nc.sync.dma_start(
                    out=st[:], in_=sT_h[:].rearrange("(kc p) b -> p kc b", p=P)
                )
                ones1 = prol.tile([1, BL], BF16, tag="ones1")
                nc.any.memset(ones1[:], 1.0)
                db = prol.tile([1, N], BF16, tag="db")
                nc.sync.dma_start(out=db[:], in_=decb_h[:])
                dwt = prol.tile([P, KC, N], BF16, tag="dwt")
                for kc in range(KC):
                    nc.sync.dma_start(
                        out=dwt[:, kc, :], in_=dwT_h[kc * P : (kc + 1) * P, :]
                    )
                for kc in range(KC):
                    nc.sync.dma_start(
                        out=wh[:, kc, :], in_=whT_h[kc * P : (kc + 1) * P, :]
                    )
                nc.sync.dma_start(out=vcol[:], in_=vcol_h[:])
                nc.sync.dma_start(out=wcT[:], in_=wcT_h[:])
                # dec_feaT[m, b] = sum_n dec_W[m, n] s_t_hat[b, n] + dec_b[m]
                for mt in range(MT):
                    msl = slice(mt * P, (mt + 1) * P)
                    ps_d = psA.tile([P, BL], F32, tag="psA")
                    for kc in range(KC):
                        nc.tensor.matmul(
                            ps_d[:, :],
                            dwt[:, kc, msl],
                            st[:, kc, :],
                            start=(kc == 0),
                            stop=False,
                        )
                    nc.tensor.matmul(
                        ps_d[:, :], db[:, msl], ones1[:],
                        start=False, stop=True,
                    )
                    nc.vector.tensor_copy(dec_feaT[:, mt, :], ps_d[:, :])

                # broadcast every batch's cov row across partitions up front
                for b in range(BL):
                    cb = bcp.tile([P, T], BF16, tag="covbc")
                    ps_cb = psA.tile([P, T], F32, tag="psA")
                    for th in range(2):
                        sl = slice(th * 512, (th + 1) * 512)
                        nc.tensor.matmul(
                            ps_cb[:, sl], ones_col[:], covb_sb[:, b, sl],
                            start=True, stop=True,
                        )
                    nc.vector.tensor_copy(cb[:], ps_cb[:])
                    cov_bc_all.append(cb)

            # ---- main loop over local batches ----
            def load_ht8(b):
                t8 = ht8p.tile([P, KC, T], FP8, tag="ht8")
                for kc in range(KC):
                    nc.sync.dma_start(
                        out=t8[:, kc, :], in_=hT8_h[b, kc * P : (kc + 1) * P, :]
                    )
                return t8

            def load_hnat(b):
                tn = hnatp.tile([P, KC, N], BF16, tag="hnat")
                for tc_ in range(KC):
                    nc.sync.dma_start(
                        out=tn[:, tc_, :],
                        in_=hnat_h[b, tc_ * P : (tc_ + 1) * P, :],
                    )
                return tn

            def load_rows(b):
                mrow = rowp.tile([1, T], F32, tag="mask")
                nc.sync.dma_start(out=mrow[:], in_=mask_h[b : b + 1, :])
                covrow = rowp.tile([1, T], F32, tag="covrow")
                nc.sync.dma_start(out=covrow[:], in_=cov_h[b : b + 1, :])
                return mrow, covrow

            # pass-B (c_t on PE) is deferred and trickled into the next
            # batch's matmul loop: each item inserts a small PE/DVE/DMA
            # chunk between pass-A groups so nothing bursts.
            pending_pass_b = []

            def issue_pass_b_one():
                if pending_pass_b:
                    pending_pass_b.pop(0)()

            ht8_q = [load_ht8(0)]
            hnat_next = load_hnat(0)
            rows_next = load_rows(0)
            for b in range(BL):
                ht8 = ht8_q.pop(0)
                hnat = hnat_next
                mrow, covrow = rows_next

                cov_bc = cov_bc_all[b]
                ps_sc = psS.tile([1, T], F32, tag="psS")

                def issue_vdot(mt_, att_, b_=b, ps_sc_=ps_sc):
                    for th in range(2):
                        sl = slice(th * 512, (th + 1) * 512)
                        nc.tensor.matmul(
                            ps_sc_[:, sl],
                            vcol[:, mt_ : mt_ + 1],
                            att_[:, sl],
                            start=(mt_ == 0),
                            stop=(mt_ == MT - 1),
                        )

                att_prev = None
                for mt in range(MT):
                    msl = slice(mt * P, (mt + 1) * P)
                    ps_att = psA.tile([P, T], F32, tag="psA")
                    for th in range(2):
                        sl = slice(th * 512, (th + 1) * 512)
                        for kcp in range(KC // 2):
                            nc.tensor.matmul(
                                ps_att[:, sl],
                                wh[:, 2 * kcp : 2 * kcp + 2, msl],
                                ht8[:, 2 * kcp : 2 * kcp + 2, sl],
                                start=(kcp == 0),
                                stop=(kcp == KC // 2 - 1),
                                perf_mode=DR,
                            )
                    # v-dot of the PREVIOUS tile goes here, one group late:
                    # its first matmul carries a PE-pipeline-drain wait
                    # (score-PSUM reuse) plus the tanh wait, both of which
                    # are absorbed by the group just issued above.
                    if att_prev is not None:
                        issue_vdot(mt - 1, att_prev)
                    # att_pre = psum + 16*W_c[m]*cov[t], evicted to SBUF by the
                    # same DVE op (frees the PSUM buffer one hop earlier than
                    # an in-place fold followed by a PSUM-reading tanh)
                    att_pre = attp.tile([P, T], BF16, tag="attp")
                    nc.vector.scalar_tensor_tensor(
                        out=att_pre[:, :], in0=cov_bc[:, :],
                        scalar=wcT[:, mt : mt + 1], in1=ps_att[:, :],
                        op0=ALU.mult, op1=ALU.add,
                    )
                    att = attp.tile([P, T], BF16, tag="att")
                    # att = tanh(att_pre/16 + dec_fea[m])  (undo the W*16
                    # scale; bias folds the dec term)
                    nc.scalar.activation(
                        att[:], att_pre[:], AF.Tanh,
                        bias=dec_feaT[:, mt, b : b + 1],
                        scale=1.0 / WSCALE,
                    )
                    att_prev = att
                    issue_pass_b_one()
                    # prefetch upcoming batches EARLY: the fp8 tile gates the
                    # next batch's first matmul group, so it is requested two
                    # batches ahead; 3 MB of h copies issued only at the
                    # batch boundary would stall the PE ~7us per batch.
                    if mt == 0 and b == 0 and BL > 1:
                        ht8_q.append(load_ht8(1))
                    elif mt == 4:
                        if b + 2 < BL:
                            ht8_q.append(load_ht8(b + 2))
                        if b + 1 < BL:
                            hnat_next = load_hnat(b + 1)
                issue_vdot(MT - 1, att_prev)

                if b + 1 < BL:
                    rows_next = load_rows(b + 1)

                # softmax over t (no max-subtraction: |score| <= ||v||_1 ~ 26)
                erow = rowp1.tile([1, T], F32, tag="erow")
                nc.scalar.activation(erow[:], ps_sc[:], AF.Exp)
                emrow = rowp1.tile([1, T], F32, tag="emrow")
                ssum = rowp1.tile([1, 1], F32, tag="ssum")
                nc.vector.scalar_tensor_tensor(
                    out=emrow[:], in0=erow[:], scalar=1.0, in1=mrow[:],
                    op0=ALU.bypass, op1=ALU.mult, accum_out=ssum[:],
                )
                rinv = rowp1.tile([1, 1], F32, tag="rinv")
                nc.vector.reciprocal(rinv[:], ssum[:])
                arow = rowp.tile([1, T], F32, tag="arow")
                nc.vector.tensor_scalar_mul(arow[:], emrow[:], rinv[:])
                nc.sync.dma_start(out=attn_out[b : b + 1, :], in_=arow[:])
                cnrow = rowp1.tile([1, T], F32, tag="cnrow")
                nc.vector.tensor_add(cnrow[:], arow[:], covrow[:])
                nc.sync.dma_start(out=cov_out[b : b + 1, :], in_=cnrow[:])
                # unnormalized exp*mask row in bf16 feeds the PE pass B;
                # 1/sum is folded into the final psum eviction so pass B
                # does not wait on recip/normalize.
                embrow = rowp1.tile([1, T], BF16, tag="embrow")
                nc.vector.tensor_copy(embrow[:], emrow[:])

                # pass B on PE: c_t[n] = sum_t attn[t] * h[t, n]
                def make_pass_b(hnat_=hnat, b_=b, embrow_=embrow, rinv_=rinv):
                    ps_box = []
                    acol_box = []

                    def atn_dma():
                        # exp*mask row -> [128, 8] columns via a DRAM bounce
                        # (a DRAM AP can supply the partition dim directly)
                        nc.sync.dma_start(
                            out=atn_bounce[b_ : b_ + 1, :], in_=embrow_[:]
                        )
                        acol = acolp.tile([P, KC], BF16, tag="acol")
                        acol_box.append(acol)
                        nc.sync.dma_start(
                            out=acol[:],
                            in_=atn_bounce[b_ : b_ + 1, :].rearrange(
                                "o (c p) -> (o p) c", p=P
                            ),
                        )

                    def pe_ct(th):
                        def run():
                            if th == 0:
                                ps_fin_new = psF.tile([1, N], F32, tag="psF")
                                ps_box.append(ps_fin_new)
                            ps_fin = ps_box[0]
                            sl = slice(th * 512, (th + 1) * 512)
                            acol = acol_box[0]
                            for tc_ in range(KC):
                                nc.tensor.matmul(
                                    ps_fin[0:1, sl],
                                    acol[:, tc_ : tc_ + 1],
                                    hnat_[:, tc_, sl],
                                    start=(tc_ == 0),
                                    stop=(tc_ == KC - 1),
                                )
                        return run

                    def ct_evict():
                        ctrow = rowp.tile([1, N], F32, tag="ctrow")
                        nc.vector.tensor_scalar_mul(
                            ctrow[:], ps_box[0][:], rinv_[:]
                        )
                        nc.sync.dma_start(
                            out=ct_out[b_ : b_ + 1, :], in_=ctrow[:]
                        )

                    return [atn_dma, pe_ct(0), pe_ct(1), ct_evict]

                pending_pass_b.extend(make_pass_b())
                issue_pass_b_one()

            # keep the PE clock warm through the last batch's softmax wait
            ps_tw = psA.tile([P, T], F32, tag="psA")
            for _ in range(10):
                nc.tensor.matmul(
                    ps_tw[:, 0:512], ones_col[:], warm_row[:],
                    start=True, stop=True,
                )

            while pending_pass_b:
                issue_pass_b_one()

    _legalize_waits(nc)
    return nc


# Walrus rejects instructions whose sync-wait count exceeds the per-opcode
# descriptor slots ("Too many sync wait commands").  Tile can emit 2-3 waits
# on matmuls/DMAs at cross-engine convergence points.  Hoist surplus waits
# onto standalone InstEventSemaphore carriers inserted just before the
# offender in the same engine stream: the engine stalls on the carrier(s),
# then issues the real instruction with a single wait.  Engine streams are
# in-order, so this is semantics-preserving.
_WAIT_SKIP_OPS = {"InstEventSemaphore"}


def _legalize_waits(nc, limit=1):
    import bass_rust

    def make_carrier(engine, wait):
        return mybir.InstNoOp(
            name=nc.get_next_instruction_name(),
            text_hint="waitfix",
            bass_nofuse=True,
            engine=engine,
            sync_info=mybir.SyncInfo(on_wait=[wait], on_update=[]),
        )

    for fn in nc.m.functions:
        for blk in fn.blocks:
            il = blk.instructions
            i = 0
            while i < len(il):
                inst = il[i]
                op = type(inst).__name__
                si = getattr(inst, "sync_info", None)
                if (
                    op in _WAIT_SKIP_OPS
                    or si is None
                    or len(si.on_wait) <= limit
                ):
                    i += 1
                    continue
                waits = list(si.on_wait)
                keep, surplus = waits[-limit:], waits[:-limit]
                carriers = [make_carrier(inst.engine, w) for w in surplus]
                inst.sync_info = bass_rust.SyncInfo(
                    on_wait=keep, on_update=si.on_update
                )
                for k, ev in enumerate(carriers):
                    il.insert(i + k, ev)
                i += len(carriers) + 1


def _get_nc():
    global _NC_CACHE
    if _NC_CACHE is None:
        _NC_CACHE = build_bass()
    return _NC_CACHE


def kernel(s_t_hat, h, enc_padding_mask, coverage, W_h, W_c, dec_W, dec_b, v):
    global LAST_EXEC_NS
    import ml_dtypes

    bf16 = ml_dtypes.bfloat16
    e4m3 = ml_dtypes.float8_e4m3  # IEEE-style: max 240, matches TRN FP8_EXP4
    s_t_hat = np.asarray(s_t_hat, dtype=np.float32)
    h = np.asarray(h, dtype=np.float32)
    enc_padding_mask = np.ascontiguousarray(
        np.asarray(enc_padding_mask, dtype=np.float32)
    )
    coverage = np.ascontiguousarray(np.asarray(coverage, dtype=np.float32))
    W_h = np.asarray(W_h, dtype=np.float32)
    W_c = np.asarray(W_c, dtype=np.float32).reshape(1, N)
    dec_W = np.asarray(dec_W, dtype=np.float32)
    dec_b = np.asarray(dec_b, dtype=np.float32).reshape(1, N)
    v = np.asarray(v, dtype=np.float32)

    # fp8 pass-A operands (W pre-scaled x16 to stay in e4m3 normal range)
    W8 = (W_h * WSCALE).astype(e4m3)
    h8 = h.astype(e4m3)
    WhT8 = np.ascontiguousarray(W8.T)  # [n, m] e4m3
    hT8 = np.ascontiguousarray(np.transpose(h8, (0, 2, 1)))  # [B, N, T] e4m3

    # rank-1 score-domain correction for the fp8 quantization error:
    #   r[b,t] = v^T (W h - Wq hq)[b,t]
    #          = (dW^T v) . hq[b,t] + ((Wq + dW)^T v) . dh[b,t]
    # scaled by CBAR ~= E[tanh'(att)] and folded into the softmax for
    # free via exp(s + rc) * mask == exp(s) * (mask * exp(rc)).
    Wq = W8.astype(np.float32) / WSCALE
    dW = W_h - Wq
    dh = h - h8.astype(np.float32)
    u = dW.T @ v
    w2 = Wq.T @ v
    r = h8.astype(np.float32).reshape(B * T, N) @ u + dh.reshape(B * T, N) @ (
        w2 + u
    )
    mask_eff = np.ascontiguousarray(
        enc_padding_mask * np.exp(CBAR * r.reshape(B, T)).astype(np.float32)
    )

    hnat = np.ascontiguousarray(h.astype(bf16))  # [B, T, N] natural layout
    decWT = np.ascontiguousarray(dec_W.T.astype(bf16))  # [n, m]
    sT = np.ascontiguousarray(s_t_hat.T.astype(bf16))  # [n, B]
    vcol = np.ascontiguousarray(v.reshape(KC, P).T.astype(bf16))  # [p, kc]
    covb = coverage.astype(bf16)
    wcT = np.ascontiguousarray(
        (W_c * WSCALE).reshape(KC, P).T.astype(np.float32)
    )  # [p, kc], pre-scaled to match the x16 PSUM
    decb_b = np.ascontiguousarray(dec_b.astype(bf16))

    in_maps = []
    for c in range(NCORES):
        bs = slice(c * BL, (c + 1) * BL)
        in_maps.append(
            {
                "hT8": hT8[bs],
                "hnat": hnat[bs],
                "cov": coverage[bs],
                "covb": covb[bs],
                "mask": mask_eff[bs],
                "sT": np.ascontiguousarray(sT[:, bs]),
                "WhT8": WhT8,
                "decWT": decWT,
                "decb": decb_b,
                "WcT": wcT,
                "vcol": vcol,
            }
        )

    nc = _get_nc()
    trace = os.environ.get("BASS_KERNEL_TRACE", "0") == "1"
    res = run_bass_kernel_spmd(
        nc, in_maps, core_ids=list(range(NCORES)), trace=trace
    )
    LAST_EXEC_NS = res.exec_time_ns

    c_t = np.concatenate([res.results[c]["out_ct"] for c in range(NCORES)], axis=0)
    attn = np.concatenate(
        [res.results[c]["out_attn"] for c in range(NCORES)], axis=0
    )
    cov_new = np.concatenate(
        [res.results[c]["out_cov"] for c in range(NCORES)], axis=0
    )
    return (c_t, attn, cov_new)


# revision 54
# speedup vs baseline: 1.1952x; 1.1952x over previous
"""Pointer-generator attention kernel for 8 TRN2 NeuronCores.

Computation (per batch b):
    enc_feat = h[b] @ W_h.T                       # [T, N]
    att      = enc_feat + dec_fea[b] + cov[b,:,None] * W_c
    scores   = tanh(att) @ v                      # [T]
    attn     = exp(scores) * mask / sum(...)      # [T]
    c_t      = attn @ h[b]                        # [N]
    cov_new  = cov + attn

Sharding: data-parallel over batch, 8 batches per core, no collectives.

Device-side layout (per core):
    Pass A runs in fp8-e4m3 with DoubleRow perf mode (2 K-planes per
    matmul): hT8 [8, N, T] e4m3 and W_hT e4m3 (W pre-scaled by 16 so its
    ~N(0, 1/1024) entries stay in e4m3's normal range; the tanh
    activation applies scale=1/16 to undo it).  PSUM accumulates fp32.
    The fp8 quantization error is compensated by a rank-1 score-domain
    correction r[b, t] ~= cbar * v^T (W h - W8 h8)[b, t] (cbar = E[tanh'])
    precomputed on the host and folded into the softmax for free via
    exp(s + rc) * mask == exp(s) * (mask * exp(rc)) -- this brings the
    output rel-err from ~2.3e-2 down to ~1.3e-2.
    The cov[t] * W_c[m] fold runs on the DVE fused with the PSUM->SBUF
    eviction; tanh on ScalarE (bf16 att), v-dot as M=1 bf16 matmuls on
    PE, softmax on single-partition rows (exp has no overflow risk:
    |score| <= ||v||_1 ~ 26).
    Pass B (c_t) runs on the PE for EVERY batch: the unnormalized
    exp*mask row is bounced through DRAM into a [128, 8] column tile
    (a DRAM AP can supply the partition dim directly), then M=1 bf16
    matmuls against h in natural [t, n] layout accumulate c_t, with
    1/sum folded into the PSUM eviction.  This keeps the DVE off the
    critical path (it was the bottleneck at ~23us/batch with a
    DVE-based pass B; the PE absorbs it for ~4us/batch).
"""

import os
import sys

import numpy as np

sys.path.insert(0, "/opt/trn_rl_repo")

import concourse.bass as bass  # noqa: E402
import concourse.tile as tile  # noqa: E402
from concourse import mybir  # noqa: E402
from concourse.bass_utils import run_bass_kernel_spmd  # noqa: E402

B, T, N = 64, 1024, 1024
NCORES = 8
BL = B // NCORES  # 8 local batches per core
P = 128
KC = N // P  # 8 contraction chunks
MT = N // P  # 8 output row tiles
F32 = mybir.dt.float32
BF16 = mybir.dt.bfloat16
FP8 = mybir.dt.float8e4
AF = mybir.ActivationFunctionType
ALU = mybir.AluOpType
DR = mybir.MatmulPerfMode.DoubleRow

WSCALE = 16.0  # W_h pre-scale before e4m3 quantization
CBAR = 0.5  # E[tanh'(att)] used by the rank-1 fp8 correction

LAST_EXEC_NS = None
_NC_CACHE = None


def build_bass():
    nc = bass.Bass()

    hT8_h = nc.declare_dram_parameter("hT8", [BL, N, T], FP8, isOutput=False)
    hnat_h = nc.declare_dram_parameter("hnat", [BL, T, N], BF16, isOutput=False)
    cov_h = nc.declare_dram_parameter("cov", [BL, T], F32, isOutput=False)
    covb_h = nc.declare_dram_parameter("covb", [BL, T], BF16, isOutput=False)
    # mask_h actually carries mask * exp(rc): the rank-1 fp8 score
    # correction rc enters the softmax for free via
    # exp(s + rc) * mask == exp(s) * (mask * exp(rc)).
    mask_h = nc.declare_dram_parameter("mask", [BL, T], F32, isOutput=False)
    sT_h = nc.declare_dram_parameter("sT", [N, BL], BF16, isOutput=False)
    whT_h = nc.declare_dram_parameter("WhT8", [N, N], FP8, isOutput=False)
    dwT_h = nc.declare_dram_parameter("decWT", [N, N], BF16, isOutput=False)
    decb_h = nc.declare_dram_parameter("decb", [1, N], BF16, isOutput=False)
    wcT_h = nc.declare_dram_parameter("WcT", [P, KC], F32, isOutput=False)
    vcol_h = nc.declare_dram_parameter("vcol", [P, KC], BF16, isOutput=False)

    atn_bounce = nc.dram_tensor("atn_bounce", [BL, T], BF16)
    ct_out = nc.declare_dram_parameter("out_ct", [BL, N], F32, isOutput=True)
    attn_out = nc.declare_dram_parameter("out_attn", [BL, T], F32, isOutput=True)
    cov_out = nc.declare_dram_parameter("out_cov", [BL, T], F32, isOutput=True)

    with tile.TileContext(nc) as tc:
        with (
            tc.tile_pool(name="const", bufs=1) as const,
            tc.tile_pool(name="ht8", bufs=3) as ht8p,
            tc.tile_pool(name="hnat", bufs=3) as hnatp,
            tc.tile_pool(name="att", bufs=3) as attp,
            tc.tile_pool(name="rows", bufs=2) as rowp,
            tc.tile_pool(name="rows1", bufs=2) as rowp1,
            tc.tile_pool(name="bc", bufs=BL) as bcp,
            tc.tile_pool(name="acol", bufs=2) as acolp,
            tc.tile_pool(name="psA", bufs=2, space="PSUM") as psA,
            tc.tile_pool(name="psS", bufs=1, space="PSUM") as psS,
            tc.tile_pool(name="psF", bufs=1, space="PSUM") as psF,
        ):
            # ---- PE warm-up: dummy matmuls while the first DMAs land, so
            # the HAM clock gate reaches 2.4 GHz before real work starts ----
            ones_col = const.tile([1, P], BF16)  # lhsT for broadcasts
            nc.any.memset(ones_col[:], 1.0)
            warm_row = const.tile([1, 512], BF16)
            nc.any.memset(warm_row[:], 0.0)
            ps_w = psA.tile([P, T], F32, tag="psA")
            for _ in range(16):
                nc.tensor.matmul(
                    ps_w[:, 0:512], ones_col[:], warm_row[:],
                    start=True, stop=True,
                )

            # ---- constants (issue order matters: prologue inputs first) ----
            wh = const.tile([P, KC, N], FP8)  # [n%128, n//128, m], W.T * 16
            vcol = const.tile([P, KC], BF16)
            wcT = const.tile([P, KC], F32)  # 16*W_c[mt*128+p] per-part scalars
            covb_sb = const.tile([1, BL, T], BF16)  # cov rows (bf16)
            nc.sync.dma_start(out=covb_sb[:], in_=covb_h[:].unsqueeze(0))
            dec_feaT = const.tile([P, MT, BL], F32)  # dec_fea[m, b] bias layout

            cov_bc_all = []

            # ---- prologue: dec_fea = s_t_hat @ dec_W.T + dec_b  -> [b, m] ----
            with tc.tile_pool(name="prol", bufs=1) as prol:
                st = prol.tile([P, KC, BL], BF16, tag="st")
                nc.sync.dma_start(
                    out=st[:], in_=sT_h[:].rearrange("(kc p) b -> p kc b", p=P)
                )
                ones1 = prol.tile([1, BL], BF16, tag="ones1")
                nc.any.memset(ones1[:], 1.0)
                db = prol.tile([1, N], BF16, tag="db")
                nc.sync.dma_start(out=db[:], in_=decb_h[:])
                dwt = prol.tile([P, KC, N], BF16, tag="dwt")
                for kc in range(KC):
                    nc.sync.dma_start(
                        out=dwt[:, kc, :], in_=dwT_h[kc * P : (kc + 1) * P, :]
                    )
                for kc in range(KC):
                    nc.sync.dma_start(
                        out=wh[:, kc, :], in_=whT_h[kc * P : (kc + 1) * P, :]
                    )
                nc.sync.dma_start(out=vcol[:], in_=vcol_h[:])
                nc.sync.dma_start(out=wcT[:], in_=wcT_h[:])
                # dec_feaT[m, b] = sum_n dec_W[m, n] s_t_hat[b, n] + dec_b[m]
                for mt in range(MT):
                    msl = slice(mt * P, (mt + 1) * P)
                    ps_d = psA.tile([P, BL], F32, tag="psA")
                    for kc in range(KC):
                        nc.tensor.matmul(
                            ps_d[:, :],
                            dwt[:, kc, msl],
                            st[:, kc, :],
                            start=(kc == 0),
                            stop=False,
                        )
                    nc.tensor.matmul(
                        ps_d[:, :], db[:, msl], ones1[:],
                        start=False, stop=True,
                    )
                    nc.vector.tensor_copy(dec_feaT[:, mt, :], ps_d[:, :])

                # broadcast every batch's cov row across partitions up front
                for b in range(BL):
                    cb = bcp.tile([P, T], BF16, tag="covbc")
                    ps_cb = psA.tile([P, T], F32, tag="psA")
                    for th in range(2):
                        sl = slice(th * 512, (th + 1) * 512)
                        nc.tensor.matmul(
                            ps_cb[:, sl], ones_col[:], covb_sb[:, b, sl],
                            start=True, stop=True,
                        )
                    nc.vector.tensor_copy(cb[:], ps_cb[:])
                    cov_bc_all.append(cb)

            # ---- main loop over local batches ----
            def load_ht8(b):
                t8 = ht8p.tile([P, KC, T], FP8, tag="ht8")
                for kc in range(KC):
                    nc.sync.dma_start(
                        out=t8[:, kc, :], in_=hT8_h[b, kc * P : (kc + 1) * P, :]
                    )
                return t8

            def load_hnat(b):
                tn = hnatp.tile([P, KC, N], BF16, tag="hnat")
                for tc_ in range(KC):
                    nc.sync.dma_start(
                        out=tn[:, tc_, :],
                        in_=hnat_h[b, tc_ * P : (tc_ + 1) * P, :],
                    )
                return tn

            def load_rows(b):
                mrow = rowp.tile([1, T], F32, tag="mask")
                nc.sync.dma_start(out=mrow[:], in_=mask_h[b : b + 1, :])
                covrow = rowp.tile([1, T], F32, tag="covrow")
                nc.sync.dma_start(out=covrow[:], in_=cov_h[b : b + 1, :])
                return mrow, covrow

            # pass-B (c_t on PE) is deferred and trickled into the next
            # batch's matmul loop: each item inserts a small PE/DVE/DMA
            # chunk between pass-A groups so nothing bursts.
            pending_pass_b = []

            def issue_pass_b_one():
                if pending_pass_b:
                    pending_pass_b.pop(0)()

            ht8_q = [load_ht8(0)]
            hnat_next = load_hnat(0)
            rows_next = load_rows(0)
            for b in range(BL):
                ht8 = ht8_q.pop(0)
                hnat = hnat_next
                mrow, covrow = rows_next

                cov_bc = cov_bc_all[b]
                ps_sc = psS.tile([1, T], F32, tag="psS")

                def issue_vdot(mt_, att_, b_=b, ps_sc_=ps_sc):
                    for th in range(2):
                        sl = slice(th * 512, (th + 1) * 512)
                        nc.tensor.matmul(
                            ps_sc_[:, sl],
                            vcol[:, mt_ : mt_ + 1],
                            att_[:, sl],
                            start=(mt_ == 0),
                            stop=(mt_ == MT - 1),
                        )

                for mt in range(MT):
                    msl = slice(mt * P, (mt + 1) * P)
                    ps_att = psA.tile([P, T], F32, tag="psA")
                    for th in range(2):
                        sl = slice(th * 512, (th + 1) * 512)
                        for kcp in range(KC // 2):
                            nc.tensor.matmul(
                                ps_att[:, sl],
                                wh[:, 2 * kcp : 2 * kcp + 2, msl],
                                ht8[:, 2 * kcp : 2 * kcp + 2, sl],
                                start=(kcp == 0),
                                stop=(kcp == KC // 2 - 1),
                                perf_mode=DR,
                            )
                    # att_pre = psum + 16*W_c[m]*cov[t], evicted to SBUF by the
                    # same DVE op (frees the PSUM buffer one hop earlier than
                    # an in-place fold followed by a PSUM-reading tanh)
                    att_pre = attp.tile([P, T], BF16, tag="attp")
                    nc.vector.scalar_tensor_tensor(
                        out=att_pre[:, :], in0=cov_bc[:, :],
                        scalar=wcT[:, mt : mt + 1], in1=ps_att[:, :],
                        op0=ALU.mult, op1=ALU.add,
                    )
                    att = attp.tile([P, T], BF16, tag="att")
                    # att = tanh(att_pre/16 + dec_fea[m])  (undo the W*16
                    # scale; bias folds the dec term)
                    nc.scalar.activation(
                        att[:], att_pre[:], AF.Tanh,
                        bias=dec_feaT[:, mt, b : b + 1],
                        scale=1.0 / WSCALE,
                    )
                    issue_vdot(mt, att)
                    issue_pass_b_one()
                    # prefetch upcoming batches EARLY: the fp8 tile gates the
                    # next batch's first matmul group, so it is requested two
                    # batches ahead; 3 MB of h copies issued only at the
                    # batch boundary would stall the PE ~7us per batch.
                    if mt == 0 and b == 0 and BL > 1:
                        ht8_q.append(load_ht8(1))
                    elif mt == 4:
                        if b + 2 < BL:
                            ht8_q.append(load_ht8(b + 2))
                        if b + 1 < BL:
                            hnat_next = load_hnat(b + 1)

                if b + 1 < BL:
                    rows_next = load_rows(b + 1)

                # softmax over t (no max-subtraction: |score| <= ||v||_1 ~ 26)
                erow = rowp1.tile([1, T], F32, tag="erow")
                nc.scalar.activation(erow[:], ps_sc[:], AF.Exp)
                emrow = rowp1.tile([1, T], F32, tag="emrow")
                ssum = rowp1.tile([1, 1], F32, tag="ssum")
                nc.vector.scalar_tensor_tensor(
                    out=emrow[:], in0=erow[:], scalar=1.0, in1=mrow[:],
                    op0=ALU.bypass, op1=ALU.mult, accum_out=ssum[:],
                )
                rinv = rowp1.tile([1, 1], F32, tag="rinv")
                nc.vector.reciprocal(rinv[:], ssum[:])
                arow = rowp.tile([1, T], F32, tag="arow")
                nc.vector.tensor_scalar_mul(arow[:], emrow[:], rinv[:])
                nc.sync.dma_start(out=attn_out[b : b + 1, :], in_=arow[:])
                cnrow = rowp1.tile([1, T], F32, tag="cnrow")
                nc.vector.tensor_add(cnrow[:], arow[:], covrow[:])
                nc.sync.dma_start(out=cov_out[b : b + 1, :], in_=cnrow[:])
                # unnormalized exp*mask row in bf16 feeds the PE pass B;
                # 1/sum is folded into the final psum eviction so pass B
                # does not wait on recip/normalize.
                embrow = rowp1.tile([1, T], BF16, tag="embrow")
                nc.vector.tensor_copy(embrow[:], emrow[:])

                # pass B on PE: c_t[n] = sum_t attn[t] * h[t, n]
                def make_pass_b(hnat_=hnat, b_=b, embrow_=embrow, rinv_=rinv):
                    ps_box = []
                    acol_box = []

                    def atn_dma():
                        # exp*mask row -> [128, 8] columns via a DRAM bounce
                        # (a DRAM AP can supply the partition dim directly)
                        nc.sync.dma_start(
                            out=atn_bounce[b_ : b_ + 1, :], in_=embrow_[:]
                        )
                        acol = acolp.tile([P, KC], BF16, tag="acol")
                        acol_box.append(acol)
                        nc.sync.dma_start(
                            out=acol[:],
                            in_=atn_bounce[b_ : b_ + 1, :].rearrange(
                                "o (c p) -> (o p) c", p=P
                            ),
                        )

                    def pe_ct(th):
                        def run():
                            if th == 0:
                                ps_fin_new = psF.tile([1, N], F32, tag="psF")
                                ps_box.append(ps_fin_new)
                            ps_fin = ps_box[0]
                            sl = slice(th * 512, (th + 1) * 512)
                            acol = acol_box[0]
                            for tc_ in range(KC):
                                nc.tensor.matmul(
                                    ps_fin[0:1, sl],
                                    acol[:, tc_ : tc_ + 1],
                                    hnat_[:, tc_, sl],
                                    start=(tc_ == 0),
                                    stop=(tc_ == KC - 1),
                                )
                        return run

                    def ct_evict():
                        ctrow = rowp.tile([1, N], F32, tag="ctrow")
                        nc.vector.tensor_scalar_mul(
                            ctrow[:], ps_box[0][:], rinv_[:]
                        )
                        nc.sync.dma_start(
                            out=ct_out[b_ : b_ + 1, :], in_=ctrow[:]
                        )

                    return [atn_dma, pe_ct(0), pe_ct(1), ct_evict]

                pending_pass_b.extend(make_pass_b())
                issue_pass_b_one()

            # keep the PE clock warm through the last batch's softmax wait
            ps_tw = psA.tile([P, T], F32, tag="psA")
            for _ in range(10):
                nc.tensor.matmul(
                    ps_tw[:, 0:512], ones_col[:], warm_row[:],
                    start=True, stop=True,
                )

            while pending_pass_b:
                issue_pass_b_one()

    _legalize_waits(nc)
    return nc


# Walrus rejects instructions whose sync-wait count exceeds the per-opcode
# descriptor slots ("Too many sync wait commands").  Tile can emit 2-3 waits
# on matmuls/DMAs at cross-engine convergence points.  Hoist surplus waits
# onto standalone InstEventSemaphore carriers inserted just before the
# offender in the same engine stream: the engine stalls on the carrier(s),
# then issues the real instruction with a single wait.  Engine streams are
# in-order, so this is semantics-preserving.
_WAIT_SKIP_OPS = {"InstEventSemaphore"}


def _legalize_waits(nc, limit=1):
    import bass_rust

    def make_carrier(engine, wait):
        return mybir.InstNoOp(
            name=nc.get_next_instruction_name(),
            text_hint="waitfix",
            bass_nofuse=True,
            engine=engine,
            sync_info=mybir.SyncInfo(on_wait=[wait], on_update=[]),
        )

    for fn in nc.m.functions:
        for blk in fn.blocks:
            il = blk.instructions
            i = 0
            while i < len(il):
                inst = il[i]
                op = type(inst).__name__
                si = getattr(inst, "sync_info", None)
                if (
                    op in _WAIT_SKIP_OPS
                    or si is None
                    or len(si.on_wait) <= limit
                ):
                    i += 1
                    continue
                waits = list(si.on_wait)
                keep, surplus = waits[-limit:], waits[:-limit]
                carriers = [make_carrier(inst.engine, w) for w in surplus]
                inst.sync_info = bass_rust.SyncInfo(
                    on_wait=keep, on_update=si.on_update
                )
                for k, ev in enumerate(carriers):
                    il.insert(i + k, ev)
                i += len(carriers) + 1


def _get_nc():
    global _NC_CACHE
    if _NC_CACHE is None:
        _NC_CACHE = build_bass()
    return _NC_CACHE


def kernel(s_t_hat, h, enc_padding_mask, coverage, W_h, W_c, dec_W, dec_b, v):
    global LAST_EXEC_NS
    import ml_dtypes

    bf16 = ml_dtypes.bfloat16
    e4m3 = ml_dtypes.float8_e4m3  # IEEE-style: max 240, matches TRN FP8_EXP4
    s_t_hat = np.asarray(s_t_hat, dtype=np.float32)
    h = np.asarray(h, dtype=np.float32)
    enc_padding_mask = np.ascontiguousarray(
        np.asarray(enc_padding_mask, dtype=np.float32)
    )
    coverage = np.ascontiguousarray(np.asarray(coverage, dtype=np.float32))
    W_h = np.asarray(W_h, dtype=np.float32)
    W_c = np.asarray(W_c, dtype=np.float32).reshape(1, N)
    dec_W = np.asarray(dec_W, dtype=np.float32)
    dec_b = np.asarray(dec_b, dtype=np.float32).reshape(1, N)
    v = np.asarray(v, dtype=np.float32)

    # fp8 pass-A operands (W pre-scaled x16 to stay in e4m3 normal range)
    W8 = (W_h * WSCALE).astype(e4m3)
    h8 = h.astype(e4m3)
    WhT8 = np.ascontiguousarray(W8.T)  # [n, m] e4m3
    hT8 = np.ascontiguousarray(np.transpose(h8, (0, 2, 1)))  # [B, N, T] e4m3

    # rank-1 score-domain correction for the fp8 quantization error:
    #   r[b,t] = v^T (W h - Wq hq)[b,t]
    #          = (dW^T v) . hq[b,t] + ((Wq + dW)^T v) . dh[b,t]
    # scaled by CBAR ~= E[tanh'(att)] and folded into the softmax for
    # free via exp(s + rc) * mask == exp(s) * (mask * exp(rc)).
    Wq = W8.astype(np.float32) / WSCALE
    dW = W_h - Wq
    dh = h - h8.astype(np.float32)
    u = dW.T @ v
    w2 = Wq.T @ v
    r = h8.astype(np.float32).reshape(B * T, N) @ u + dh.reshape(B * T, N) @ (
        w2 + u
    )
    mask_eff = np.ascontiguousarray(
        enc_padding_mask * np.exp(CBAR * r.reshape(B, T)).astype(np.float32)
    )

    hnat = np.ascontiguousarray(h.astype(bf16))  # [B, T, N] natural layout
    decWT = np.ascontiguousarray(dec_W.T.astype(bf16))  # [n, m]
    sT = np.ascontiguousarray(s_t_hat.T.astype(bf16))  # [n, B]
    vcol = np.ascontiguousarray(v.reshape(KC, P).T.astype(bf16))  # [p, kc]
    covb = coverage.astype(bf16)
    wcT = np.ascontiguousarray(
        (W_c * WSCALE).reshape(KC, P).T.astype(np.float32)
    )  # [p, kc], pre-scaled to match the x16 PSUM
    decb_b = np.ascontiguousarray(dec_b.astype(bf16))

    in_maps = []
    for c in range(NCORES):
        bs = slice(c * BL, (c + 1) * BL)
        in_maps.append(
            {
                "hT8": hT8[bs],
                "hnat": hnat[bs],
                "cov": coverage[bs],
                "covb": covb[bs],
                "mask": mask_eff[bs],
                "sT": np.ascontiguousarray(sT[:, bs]),
                "WhT8": WhT8,
                "decWT": decWT,
                "decb": decb_b,
                "WcT": wcT,
                "vcol": vcol,
            }
        )

    nc = _get_nc()
    trace = os.environ.get("BASS_KERNEL_TRACE", "0") == "1"
    res = run_bass_kernel_spmd(
        nc, in_maps, core_ids=list(range(NCORES)), trace=trace
    )
    LAST_EXEC_NS = res.exec_time_ns

    c_t = np.concatenate([res.results[c]["out_ct"] for c in range(NCORES)], axis=0)
    attn = np.concatenate(
        [res.results[c]["out_attn"] for c in range(NCORES)], axis=0
    )
    cov_new = np.concatenate(
        [res.results[c]["out_cov"] for c in range(NCORES)], axis=0
    )
    return (c_t, attn, cov_new)
